# revision 5
# baseline (speedup 1.0000x reference)
"""Multi-head attention forward, distributed over 8 TRN2 NeuronCores.

Problem: x[2,2048,1024] -> QKV proj (16 heads x 64) -> softmax attention
-> output proj + bias -> [2,2048,1024], f32 I/O, bf16 tensor-engine compute.

Sharding: rows = flattened (batch, seq) = 4096 rows; core c owns rows
[c*512, (c+1)*512) -- cores 0-3 hold batch 0, cores 4-7 batch 1. Each core
projects Q/K/V for its own rows and computes attention for its 512 query
rows over all 2048 keys of its batch, then the output projection.

v2 schedule (local-first, self-skipping gathers):
  - K^T is AllGathered first (1MB in -> 4MB out), then V in two halves.
    A 16-byte dummy AllGather at kernel start absorbs the ~23us collective
    init barrier during the prologue.
  - Attention starts on the core's own 4 local key-tiles (K^T/V straight
    from SBUF, no DRAM roundtrip) while the K AllGather is in flight.
  - Remote K^T/V are unpacked with dynamic-offset DMAs using
    partition_id: slot s reads rank (pid%4 + 1 + s) % 4, skipping self.
  - Each V AllGather is triggered (gpsimd queue order) only after the
    previous gather's unpack DMAs, so wire traffic never starves the
    unpack queue (the v1 kernel lost ~40us to exactly that).
  - Remote attention pass 1 covers each rank's first 2 key-tiles (V half
    1), pass 2 the other 2 (V half 2).

Layouts (no transposes in the attention loop; every matmul contracts over
K=128 -- K=64 matmuls stream at half rate on TRN2, measured 454 vs 249ns):
  K^T/Q^T [hd, rows]  = W^T x^T; head PAIRS share a 128-partition tile.
  qTe/qTo [128, rows] Q^T of the even/odd head of the pair with the other
                      head's rows zeroed: scores lhsT is the full
                      [128, 128] K-pair tile, zero rows kill the
                      wrong-head term => full-rate K=128 matmul.
  S^T     [keys, q]   keys on partitions; exp on ACT engine ([128,1024]
                      per (pair, key-tile), the ~1.1us/instr ACT stream is
                      the pacing engine).
  att^T   [hd+1, q]   = (V | 1).T @ P^T accumulated in PSUM per pass; row
                      64 is the softmax denominator. Summed into bf16 SBUF
                      acc across passes.
  out     [rows, D]   = lhsT(attT).T @ Wo natural (+ ones-row x bo).
exp has no max subtraction (scores ~N(0,1) after the 1/sqrt(64) scale
folded into the ACT activation scale).

Host-side prep: x^T and weights are pre-transposed/cast to bf16 (halves
DMA bytes; the kernel would cast anyway).
"""

import ml_dtypes
import numpy as np

import concourse.bass as bass
import concourse.mybir as mybir
import concourse.tile as tile
from concourse import bacc
from concourse.bass_utils import run_bass_kernel_spmd

BF = mybir.dt.bfloat16
F32 = mybir.dt.float32
P = 128

N_CORES = 8
GROUP = 4  # cores per batch group (one AllGather group)


class Cfg:
    def __init__(self, rpc, d, n_heads, head_dim):
        self.RPC = rpc            # query rows per core
        self.D = d                # model dim
        self.H = n_heads
        self.HD = head_dim
        assert n_heads * head_dim == d
        self.NT_D = d // P        # dim tiles (= head pairs)
        self.NT_R = rpc // P      # row tiles (= local key tiles)
        self.KEYS = rpc * GROUP   # keys per batch group
        self.NT_K = self.KEYS // P
        assert P // head_dim == 2


FULL = Cfg(rpc=512, d=1024, n_heads=16, head_dim=64)


def _body(tc, nc, cfg, xT_in, wq_in, wk_in, wv_in, wo_in, bo_in, out_ext):
    c = cfg
    AF = mybir.ActivationFunctionType
    rg = [list(range(GROUP)), list(range(GROUP, 2 * GROUP))]
    HD1 = c.HD + 1
    NREM = GROUP - 1              # remote ranks
    RKT = c.NT_R // 2             # key tiles per V gather half (2)
    from contextlib import ExitStack

    stack = ExitStack()
    dram = stack.enter_context(tc.tile_pool(name="dram", bufs=1, space="DRAM"))
    const = stack.enter_context(tc.tile_pool(name="const", bufs=1))
    persist = stack.enter_context(tc.tile_pool(name="persist", bufs=1))

    # DRAM bounce buffers for the collectives
    dummy_in = dram.tile([1, 16], BF, name="dummy_in")
    dummy_g = dram.tile([GROUP, 16], BF, name="dummy_g")
    k_in = dram.tile([c.D, c.RPC], BF, name="k_in")
    k_g = dram.tile([GROUP * c.D, c.RPC], BF, name="k_g")
    v_in = [dram.tile([RKT * P, c.D], BF, name=f"v_in{h}") for h in range(2)]
    v_g = [dram.tile([GROUP * RKT * P, c.D], BF, name=f"v_g{h}") for h in range(2)]

    ones_row = const.tile([1, P], BF, tag="ones_row", name="ones_row")
    nc.vector.memset(ones_row[:], 1.0)
    bo_sb = const.tile([1, c.D], BF, tag="bo", name="bo_sb")
    nc.sync.dma_start(bo_sb[:], bo_in[:, :])

    def ptiles(shape, dt_, pfx, n, pool=None):
        pool = pool or persist
        return [pool.tile(shape, dt_, tag=f"{pfx}{t}", name=f"{pfx}{t}") for t in range(n)]

    xT = ptiles([P, c.RPC], BF, "xT", c.NT_D)
    kt_loc = ptiles([P, c.RPC], BF, "ktl", c.NT_D)     # local K^T per pair
    kt_rem = ptiles([P, NREM * c.RPC], BF, "ktr", c.NT_D)  # remote K^T per pair
    qTe = ptiles([P, c.RPC], BF, "qTe", c.NT_D)
    qTo = ptiles([P, c.RPC], BF, "qTo", c.NT_D)
    vloc = ptiles([P, c.D], BF, "vloc", c.NT_R)        # local V row-tiles
    # v_aug[j]: [128 keys, 16 heads x 65] with ones column per head.
    # j: 0..NT_R local, then per slot s: NT_R tiles.
    v_aug = ptiles([P, c.H * HD1], BF, "va", c.NT_K)
    attT = ptiles([P, c.RPC], BF, "attT", c.NT_D)
    wo_sb = ptiles([P, c.D], BF, "wo", c.NT_D)
    # acc[p]: [65, 2*RPC] bf16, even head cols 0:RPC, odd RPC:2RPC
    acc = ptiles([HD1, 2 * c.RPC], BF, "acc", c.NT_D)

    def emit_ag(ins, outs):
        nc.gpsimd.collective_compute(
            "AllGather",
            mybir.AluOpType.bypass,
            replica_groups=rg,
            ins=[ins[:].opt()],
            outs=[outs[:].opt()],
        )

    # absorb the collective init barrier while the prologue runs
    nc.sync.dma_start(dummy_in[:], bo_in[:, 0:16])
    emit_ag(dummy_in, dummy_g)

    with (
        tc.tile_pool(name="wpool", bufs=1) as wpool,
        tc.tile_pool(name="proj_psum", bufs=4, space="PSUM") as proj_psum,
    ):
        wk_sb = ptiles([P, c.D], BF, "wk", c.NT_D, pool=wpool)
        wq_sb = ptiles([P, c.D], BF, "wq", c.NT_D, pool=wpool)
        wv_sb = ptiles([P, c.D], BF, "wv", c.NT_D, pool=wpool)
        # ---- loads: x first, then weights in use order ----
        for t in range(c.NT_D):
            nc.sync.dma_start(xT[t][:], xT_in[t * P : (t + 1) * P, :])
        for t in range(c.NT_D):
            nc.sync.dma_start(wk_sb[t][:], wk_in[t * P : (t + 1) * P, :])
        for t in range(c.NT_D):
            nc.sync.dma_start(wq_sb[t][:], wq_in[t * P : (t + 1) * P, :])
        for t in range(c.NT_D):
            nc.sync.dma_start(wv_sb[t][:], wv_in[t * P : (t + 1) * P, :])
        for t in range(c.NT_D):
            nc.sync.dma_start(wo_sb[t][:], wo_in[t * P : (t + 1) * P, :])

        # ---- K^T projection -> kt_loc, pack + AllGather ----
        for m in range(c.NT_D):
            ps = proj_psum.tile([P, c.RPC], F32, tag="proj", name="proj_ps")
            for k in range(c.NT_D):
                nc.tensor.matmul(
                    ps[:],
                    wk_sb[k][:, m * P : (m + 1) * P],
                    xT[k][:],
                    start=(k == 0),
                    stop=(k == c.NT_D - 1),
                )
            nc.vector.tensor_copy(kt_loc[m][:], ps[:])
            nc.sync.dma_start(k_in[m * P : (m + 1) * P, :], kt_loc[m][:])
        emit_ag(k_in, k_g)

        # ---- Q^T projection with zero-padded even/odd variants ----
        for m in range(c.NT_D):
            ps = proj_psum.tile([P, c.RPC], F32, tag="proj", name="proj_ps")
            for k in range(c.NT_D):
                nc.tensor.matmul(
                    ps[:],
                    wq_sb[k][:, m * P : (m + 1) * P],
                    xT[k][:],
                    start=(k == 0),
                    stop=(k == c.NT_D - 1),
                )
            nc.vector.tensor_copy(qTe[m][0 : c.HD, :], ps[0 : c.HD, :])
            nc.vector.memset(qTe[m][c.HD : P, :], 0.0)
            nc.vector.memset(qTo[m][0 : c.HD, :], 0.0)
            nc.vector.tensor_copy(qTo[m][c.HD : P, :], ps[c.HD : P, :])

        # ---- V projection -> vloc, pack halves for the V gathers ----
        chunk = 512
        for rt in range(c.NT_R):
            for n in range(c.D // chunk):
                ps = proj_psum.tile([P, chunk], F32, tag="proj", name="proj_ps")
                for k in range(c.NT_D):
                    nc.tensor.matmul(
                        ps[:],
                        xT[k][:, rt * P : (rt + 1) * P],
                        wv_sb[k][:, n * chunk : (n + 1) * chunk],
                        start=(k == 0),
                        stop=(k == c.NT_D - 1),
                    )
                nc.vector.tensor_copy(
                    vloc[rt][:, n * chunk : (n + 1) * chunk], ps[:]
                )
            h, lrt = divmod(rt, RKT)
            nc.sync.dma_start(
                v_in[h][lrt * P : (lrt + 1) * P, :], vloc[rt][:]
            )

    def build_v_aug(j, src):
        nc.vector.tensor_copy(
            v_aug[j][:].rearrange("p (h e) -> p h e", e=HD1)[:, :, 0 : c.HD],
            src.rearrange("p (h e) -> p h e", e=c.HD),
        )
        ones_col = v_aug[j][:].rearrange("p (h e) -> p h e", e=HD1)[:, :, c.HD : HD1]
        nc.vector.memset(ones_col, 1.0)

    for j in range(c.NT_R):
        build_v_aug(j, vloc[j][:])

    rank = nc.sync.partition_id() % GROUP

    with (
        tc.tile_pool(name="vstage", bufs=6) as vstage,
        tc.tile_pool(name="pT", bufs=10) as pT_pool,
        tc.tile_pool(name="small", bufs=4) as small,
        tc.tile_pool(name="sc_psum", bufs=3, space="PSUM") as sc_psum,
        tc.tile_pool(name="att_psum", bufs=1, space="PSUM") as att_psum,
    ):
        # attention over a list of (kt-tile AP provider, v_aug index) pairs;
        # accumulates att^T into acc[p] (copy on first pass, add after).
        def att_pass(kts, first, last):
            for p in range(c.NT_D):
                he, ho = 2 * p, 2 * p + 1
                att = att_psum.tile([HD1, 2 * c.RPC], F32, tag="att", name="att")
                for idx, (kt_ap, j) in enumerate(kts):
                    sc = sc_psum.tile([P, 2 * c.RPC], F32, tag="sc", name="sc")
                    kt = kt_ap(p)
                    nc.tensor.matmul(
                        sc[:, 0 : c.RPC], kt, qTe[p][:], start=True, stop=True
                    )
                    nc.tensor.matmul(
                        sc[:, c.RPC : 2 * c.RPC], kt, qTo[p][:], start=True, stop=True
                    )
                    pT = pT_pool.tile([P, 2 * c.RPC], BF, tag="pT", name="pT")
                    nc.scalar.activation(
                        pT[:], sc[:], AF.Exp, scale=1.0 / float(np.sqrt(c.HD))
                    )
                    nc.tensor.matmul(
                        att[:, 0 : c.RPC],
                        v_aug[j][:, he * HD1 : (he + 1) * HD1],
                        pT[:, 0 : c.RPC],
                        start=(idx == 0),
                        stop=(idx == len(kts) - 1),
                    )
                    nc.tensor.matmul(
                        att[:, c.RPC : 2 * c.RPC],
                        v_aug[j][:, ho * HD1 : (ho + 1) * HD1],
                        pT[:, c.RPC : 2 * c.RPC],
                        start=(idx == 0),
                        stop=(idx == len(kts) - 1),
                    )
                if first:
                    nc.vector.tensor_copy(acc[p][:], att[:])
                else:
                    nc.vector.tensor_add(acc[p][:], att[:], acc[p][:])

                if last:
                    den = small.tile([1, 2 * c.RPC], F32, tag="den", name="den", bufs=2)
                    nc.vector.tensor_copy(den[:], acc[p][c.HD : HD1, :])
                    rcp = small.tile([1, 2 * c.RPC], F32, tag="rcp", name="rcp", bufs=2)
                    nc.vector.reciprocal_approx_fast(rcp[:], den[:])
                    rcpb = small.tile([c.HD, 2 * c.RPC], F32, tag="rcpb", name="rcpb", bufs=2)
                    nc.gpsimd.partition_broadcast(rcpb[:], rcp[:])
                    nc.vector.tensor_mul(
                        attT[p][0 : c.HD, :], acc[p][0 : c.HD, 0 : c.RPC], rcpb[:, 0 : c.RPC]
                    )
                    nc.vector.tensor_mul(
                        attT[p][c.HD : P, :],
                        acc[p][0 : c.HD, c.RPC : 2 * c.RPC],
                        rcpb[:, c.RPC : 2 * c.RPC],
                    )

        # ---- local attention (own 4 key tiles, straight from SBUF) ----
        local_kts = [
            (lambda p, _j=j: kt_loc[p][:, _j * P : (_j + 1) * P], j)
            for j in range(c.NT_R)
        ]
        att_pass(local_kts, first=True, last=False)

        # ---- unpack remote K^T (dynamic offsets skip own rank) ----
        for s in range(NREM):
            srcrank = (rank + 1 + s) % GROUP
            for m in range(c.NT_D):
                nc.sync.dma_start(
                    kt_rem[m][:, s * c.RPC : (s + 1) * c.RPC],
                    k_g[:, :][bass.ts(srcrank * c.NT_D + m, P), :],
                )

        # V gathers are triggered only now (gpsimd program order) so their
        # wire traffic cannot starve the K unpack DMAs above. The gpsimd
        # copy below forces the trigger to wait for the last K unpack.
        ksync = small.tile([1, 16], BF, tag="ksync", name="ksync", bufs=1)
        nc.gpsimd.tensor_copy(ksync[:], kt_rem[c.NT_D - 1][0:1, (NREM - 1) * c.RPC : (NREM - 1) * c.RPC + 16])
        emit_ag(v_in[0], v_g[0])
        emit_ag(v_in[1], v_g[1])

        # ---- remote passes: half h covers key tiles {h*RKT..} of each rank ----
        for h in range(2):
            for s in range(NREM):
                srcrank = (rank + 1 + s) % GROUP
                for lrt in range(RKT):
                    vst = vstage.tile([P, c.D], BF, tag="vst", name="vst")
                    nc.sync.dma_start(
                        vst[:],
                        v_g[h][:, :][bass.ts(srcrank * RKT + lrt, P), :],
                    )
                    build_v_aug(c.NT_R + s * c.NT_R + h * RKT + lrt, vst[:])
            kts = [
                (
                    lambda p, _s=s, _j=j: kt_rem[p][
                        :, _s * c.RPC + (h * RKT + _j) * P : _s * c.RPC + (h * RKT + _j + 1) * P
                    ],
                    c.NT_R + s * c.NT_R + h * RKT + j,
                )
                for s in range(NREM)
                for j in range(RKT)
            ]
            att_pass(kts, first=False, last=(h == 1))

        # ---- output projection + bias ----
        for rt in range(c.NT_R):
            out_sb = small.tile([P, c.D], F32, tag="outsb", name="outsb", bufs=2)
            for n in range(c.D // 512):
                po = sc_psum.tile([P, 2 * c.RPC], F32, tag="sc", name="po")
                for k in range(c.NT_D):
                    nc.tensor.matmul(
                        po[:, 0:512],
                        attT[k][:, rt * P : (rt + 1) * P],
                        wo_sb[k][:, n * 512 : (n + 1) * 512],
                        start=(k == 0),
                        stop=False,
                    )
                nc.tensor.matmul(
                    po[:, 0:512],
                    ones_row[:],
                    bo_sb[:, n * 512 : (n + 1) * 512],
                    start=False,
                    stop=True,
                )
                nc.vector.tensor_copy(out_sb[:, n * 512 : (n + 1) * 512], po[:, 0:512])
            nc.sync.dma_start(out_ext[rt * P : (rt + 1) * P, :], out_sb[:])

    stack.close()


def build_nc(cfg):
    nc = bacc.Bacc(
        "TRN2", target_bir_lowering=False, debug=False, num_devices=N_CORES
    )
    c = cfg
    xT_in = nc.dram_tensor("xT", [c.D, c.RPC], BF, kind="ExternalInput")
    wq_in = nc.dram_tensor("Wq", [c.D, c.D], BF, kind="ExternalInput")
    wk_in = nc.dram_tensor("Wk", [c.D, c.D], BF, kind="ExternalInput")
    wv_in = nc.dram_tensor("Wv", [c.D, c.D], BF, kind="ExternalInput")
    wo_in = nc.dram_tensor("Wo", [c.D, c.D], BF, kind="ExternalInput")
    bo_in = nc.dram_tensor("bo", [1, c.D], BF, kind="ExternalInput")
    out_ext = nc.dram_tensor("out", [c.RPC, c.D], F32, kind="ExternalOutput")

    with tile.TileContext(nc) as tc:
        _body(
            tc, nc, cfg,
            xT_in.ap(), wq_in.ap(), wk_in.ap(), wv_in.ap(), wo_in.ap(),
            bo_in.ap(), out_ext.ap(),
        )
    nc.compile()
    return nc


_cached_nc = None


def _bf16(a):
    return np.ascontiguousarray(np.asarray(a, dtype=np.float32)).astype(
        ml_dtypes.bfloat16
    )


def prep_in_maps(c, x, Wq, Wk, Wv, Wo, bo):
    xf = np.ascontiguousarray(np.asarray(x, dtype=np.float32)).reshape(-1, c.D)
    wq, wk, wv, wo = _bf16(Wq), _bf16(Wk), _bf16(Wv), _bf16(Wo)
    bob = _bf16(bo).reshape(1, c.D)
    return [
        {
            "xT": np.ascontiguousarray(
                xf[cid * c.RPC : (cid + 1) * c.RPC].T.astype(ml_dtypes.bfloat16)
            ),
            "Wq": wq, "Wk": wk, "Wv": wv, "Wo": wo, "bo": bob,
        }
        for cid in range(N_CORES)
    ]


def kernel(x, Wq, Wk, Wv, Wo, bo):
    global _cached_nc
    c = FULL
    if _cached_nc is None:
        _cached_nc = build_nc(c)
    nc = _cached_nc

    in_maps = prep_in_maps(c, x, Wq, Wk, Wv, Wo, bo)
    res = run_bass_kernel_spmd(nc, in_maps, list(range(N_CORES)))
    out = np.concatenate([res.results[cid]["out"] for cid in range(N_CORES)], axis=0)
    return out.reshape(np.asarray(x).shape).astype(np.float32)
